# revision 16
# baseline (speedup 1.0000x reference)
"""Trainium2 Bass kernel for nn_ConstrainNet (block-banded dynamics residual).

Reference computation (n_state=64, n_input=32, n_all=96, T=128):
    V = net_input.reshape(T, 96)
    out block 0      = V[0, :64] - x0
    out block t+1    = [A B] @ V[t] - V[t+1, :64]        (t = 0..T-2)
    output = concat of the 128 blocks -> (8192,) f32

Sharding: time axis split across 8 NeuronCores; core k computes output
blocks t in [16k, 16k+16). Inputs arrive FULL on host, so the one-step
"halo" is just an overlapping host-side slice — no collectives needed.

The whole per-core computation is ONE augmented matmul with contraction
K = 96 + 1 + 16 = 113:
    out[j, s] = sum_a lhsT[a, j] * rhs[a, s]
      rows  0..95 : lhsT = Vm^T, rhs = [A B]^T          -> AB @ Vm[j]
      row     96  : identity-block fixup (core 0 only):
                    lhsT[96, 0] = 1, rhs[96, :] = V[0, :64]
      rows 97..112: lhsT[97+j', j] = -delta(j', j), rhs[97+j] = S[j]
                    -> subtracts S[j] (= V[t+1, :64]; x0 for block 0)

Device-side tuning relative to the fp32 baseline (12.3us measured):
  * bf16 operands (graded tolerance is 2e-2; bf16 lands ~1e-3): halves
    the input bytes and keeps the PE single-pass (fp32 matmuls run as
    LOW/HIGH double-pumped pairs, so 4 bf16 matmuls cost the same
    instruction count as the baseline's 2 fp32 matmuls).
  * FOUR K-rows packed per SBUF partition -> w[29, 320] bf16 with 640B
    per-partition DMA elements. Elements >= 512B dodge the DMA
    read-modify-write penalty (2x per-element latency below 512B), and
    29 partitions split 15/14 across the TWO hardware DGE queues (Sync
    and Scalar engines) land in one descriptor per queue with both
    descriptor generations running in parallel (~430ns transfer vs
    ~950ns for the baseline's 57x640B single-queue transfer).
    Matmul i contracts col-group i (bf16 cols [80i, 80i+80)): partition
    p of group i holds K-row 4p+i; rows >= 113 are zero padding.
  * The output store's descriptor generation is gated on the INPUT DMA
    semaphore (dma_a >= 32), not on the matmuls: desc-gen (~660ns) and
    the HWDGE ring launch (~660ns) then overlap the matmul+copy chain
    (~950ns after the input lands), with the engines reading o_t a
    measured ~300ns after the DVE copy retires.

Raw Bass (no TileContext): this walrus build rejects instructions that
carry more than one sync wait, and Tile's end-of-context drain
aggregates one wait per live semaphore. The manual chain below carries
at most one wait per instruction.
"""

import numpy as np

N_STATE = 64
N_INPUT = 32
N_ALL = N_STATE + N_INPUT  # 96
T_FULL = 128
N_CORES = 8
TB = T_FULL // N_CORES  # 16 output blocks per core
K = N_ALL + 1 + TB  # 113 contraction rows
W_COLS = N_STATE + TB  # 80: [rhs | lhsT] packed along the free dim
RPP = 4  # K-rows packed per SBUF partition (640B bf16 elements).
# RPP=6 (960B elems, 19 partitions, 6 matmuls) was tried: matmuls slow
# to 283ns each and the DMA packet stream spacing grows with element
# size — net +1.6us. 640B is the sweet spot.
KP = 29  # packed partitions: K-rows 4p..4p+3 share partition p
KPA = 24  # partitions on the Scalar HWDGE queue (Sync tail: 5)

_PROGRAM_CACHE = {}


def _build_program():
    import concourse.bass as bass
    import concourse.mybir as mybir

    f32 = mybir.dt.float32
    bf16 = mybir.dt.bfloat16
    nc = bass.Bass("TRN2", debug=False)

    w_a = nc.dram_tensor("w_a", [KPA, RPP * W_COLS], bf16, kind="ExternalInput")
    w_b = nc.dram_tensor("w_b", [KP - KPA, RPP * W_COLS], bf16, kind="ExternalInput")
    out_d = nc.dram_tensor("out", [TB, N_STATE], f32, kind="ExternalOutput")

    # Instructions are emitted straight into the main block (no nc.Block()):
    # the per-engine branch into a Block basic block costs ~400ns on the
    # critical path. Each engine executes only its own instructions, in
    # program order, so the semaphore chain below is unchanged.
    with (
        nc.sbuf_tensor([KP, RPP * W_COLS], bf16) as w_t,
        nc.psum_tensor([TB, N_STATE], f32) as acc,
        nc.sbuf_tensor([TB, N_STATE], f32) as o_t,
        nc.semaphore("dma_a") as dma_a,
        nc.semaphore("dma_b") as dma_b,
        nc.semaphore("mm") as mm,
        nc.semaphore("dma_out") as dma_out,
    ):
        # Input split 24/5: the big share on Scalar (its compiler
        # preamble retires ~150ns before Sync's), a 5-partition tail on
        # Sync whose desc-gen serializes behind Scalar's on the shared
        # HWDGE device but still lands in time for the PE. Separate
        # semaphores so the store below can gate on the big share alone.
        nc.scalar.dma_start(out=w_t[0:KPA, :], in_=w_a[:]).then_inc(dma_a, 16)
        nc.sync.dma_start(out=w_t[KPA:KP, :], in_=w_b[:]).then_inc(dma_b, 16)

        nc.tensor.wait_ge(dma_a, 16)
        nc.tensor.wait_ge(dma_b, 16)
        for i in range(RPP):
            inst = nc.tensor.matmul(
                acc[:],
                w_t[0:KP, W_COLS * i + N_STATE : W_COLS * (i + 1)],
                w_t[0:KP, W_COLS * i : W_COLS * i + N_STATE],
                start=(i == 0),
                stop=(i == RPP - 1),
            )
            if i == RPP - 1:
                inst.then_inc(mm, 1)
        nc.vector.wait_ge(mm, 1)
        nc.vector.tensor_copy(o_t[:], acc[:])
        # Store desc-gen gated on the input DMA, not on mm: descriptors
        # encode addresses only, and desc-gen (~660ns) + ring launch
        # (~660ns) comfortably cover the matmul+copy chain (~1200ns from
        # the same trigger), so the engines read o_t a measured ~500ns
        # after the copy retires. This pulls the whole store launch
        # latency off the tail.
        # Gate the store's desc-gen on the BIG input share only: desc-gen
        # (~700ns) + ring launch (~650ns) from that trigger still put the
        # first o_t read ~400ns after the copy retires (which needs the
        # full input + matmuls + copy, ~1220ns after the LAST descriptor).
        nc.sync.wait_ge(dma_a, 16)
        nc.sync.dma_start(out=out_d[:], in_=o_t[:]).then_inc(dma_out, 16)
        # (the dma_out completion sem is never waited on by the program;
        # the runtime quiesces DMA queues before output readback)

    _hoist_input_dmas(nc)
    return nc


def _hoist_input_dmas(nc):
    """Move the two input InstDMACopy to the head of the instruction list.

    Bass's constructor pre-emits per-engine register inits, const-AP
    memsets, and a final all-engine barrier before the kernel body. The
    input loads depend on none of that — their APs are static and their
    SBUF destination is untouched by the preamble — so hoisting them lets
    descriptor generation and the ~700ns HWDGE ring launch run UNDER the
    preamble instead of after it, landing the input ~1.1us earlier. Each
    engine executes its own instructions in list order, so only the
    per-engine relative order changes; every consumer of the data remains
    gated on the dma_a semaphore.
    """
    blk = nc.m.functions[0].blocks[0]
    insts = blk.instructions
    # the input loads are the first two DMA copies in program order (the
    # store is emitted last)
    dmas = [i for i in insts if type(i).__name__ == "InstDMACopy"][:2]
    assert len(dmas) == 2
    names = {i.name for i in dmas}
    rest = [i for i in insts if i.name not in names]
    blk.instructions = [rest[0], *dmas, *rest[1:]]
    got = [type(i).__name__ for i in blk.instructions[:3]]
    assert got[1] == "InstDMACopy" and got[2] == "InstDMACopy", got


def _get_program():
    if "nc" not in _PROGRAM_CACHE:
        _PROGRAM_CACHE["nc"] = _build_program()
    return _PROGRAM_CACHE["nc"]


def _make_in_maps(A, B, x0, net_input):
    import ml_dtypes

    A = np.ascontiguousarray(A, dtype=np.float32)
    B = np.ascontiguousarray(B, dtype=np.float32)
    x0 = np.ascontiguousarray(x0, dtype=np.float32)
    V = np.ascontiguousarray(net_input, dtype=np.float32).reshape(T_FULL, N_ALL)

    ab_t = np.concatenate([A, B], axis=1).T  # (96, 64)

    in_maps = []
    for k in range(N_CORES):
        w = np.zeros((K, W_COLS), dtype=np.float32)
        rhs = w[:, :N_STATE]
        lhsT = w[:, N_STATE:]
        rhs[:N_ALL] = ab_t
        # rows 97..112: -I in lhsT, S rows in rhs
        lhsT[N_ALL + 1 :] = -np.eye(TB, dtype=np.float32)
        t0 = k * TB
        if k == 0:
            rhs[N_ALL] = V[0, :N_STATE]  # identity-block fixup
            lhsT[N_ALL, 0] = 1.0
            lhsT[:N_ALL, 1:] = V[0 : TB - 1].T
            rhs[N_ALL + 1] = x0
            rhs[N_ALL + 2 :] = V[1:TB, :N_STATE]
        else:
            lhsT[:N_ALL] = V[t0 - 1 : t0 + TB - 1].T
            rhs[N_ALL + 1 :] = V[t0 : t0 + TB, :N_STATE]
        # pack RPP K-rows per partition: partition p, col-group i <- row RPP*p+i
        w4 = np.zeros((KP * RPP, W_COLS), dtype=np.float32)
        w4[:K] = w
        w4 = w4.reshape(KP, RPP * W_COLS).astype(ml_dtypes.bfloat16)
        in_maps.append({"w_a": np.ascontiguousarray(w4[:KPA]),
                        "w_b": np.ascontiguousarray(w4[KPA:])})
    return in_maps


def kernel(A, B, x0, net_input, T):
    assert int(T) == T_FULL, f"kernel hardcoded for T={T_FULL}, got {T}"
    from concourse.bass_utils import run_bass_kernel_spmd

    nc = _get_program()
    in_maps = _make_in_maps(A, B, x0, net_input)
    res = run_bass_kernel_spmd(nc, in_maps, core_ids=list(range(N_CORES)))
    out = np.concatenate([np.asarray(r["out"]).reshape(-1) for r in res.results])
    return out.astype(np.float32)


# revision 17
# speedup vs baseline: 1.1397x; 1.1397x over previous
"""Trainium2 Bass kernel for nn_ConstrainNet (block-banded dynamics residual).

Reference computation (n_state=64, n_input=32, n_all=96, T=128):
    V = net_input.reshape(T, 96)
    out block 0      = V[0, :64] - x0
    out block t+1    = [A B] @ V[t] - V[t+1, :64]        (t = 0..T-2)
    output = concat of the 128 blocks -> (8192,) f32

Sharding: time axis split across 8 NeuronCores; core k computes output
blocks t in [16k, 16k+16). Inputs arrive FULL on host, so the one-step
"halo" is just an overlapping host-side slice — no collectives needed.

The whole per-core computation is ONE augmented matmul with contraction
K = 96 + 1 + 16 = 113:
    out[j, s] = sum_a lhsT[a, j] * rhs[a, s]
      rows  0..95 : lhsT = Vm^T, rhs = [A B]^T          -> AB @ Vm[j]
      row     96  : identity-block fixup (core 0 only):
                    lhsT[96, 0] = 1, rhs[96, :] = V[0, :64]
      rows 97..112: lhsT[97+j', j] = -delta(j', j), rhs[97+j] = S[j]
                    -> subtracts S[j] (= V[t+1, :64]; x0 for block 0)

Device-side tuning relative to the fp32 single-queue baseline (12.3us):

  * bf16 operands (graded tolerance is 2e-2; bf16 lands ~2.3e-3):
    halves the input bytes, and fp32 matmuls run LOW/HIGH double-pumped
    on the PE so 4 bf16 matmuls cost the same instruction count as the
    baseline's 2 fp32 matmuls (and finish ~200ns sooner).

  * FOUR K-rows per SBUF partition -> w[29, 320] bf16 with 640B DMA
    elements (>= 512B dodges the DMA read-modify-write penalty; bigger
    960B elements slowed the matmuls and the packet stream — measured).
    Matmul i contracts col-group i (bf16 cols [80i, 80i+80)); partition
    p of group i holds K-row 4p+i; rows >= 113 are zero padding.

  * The input load is hoisted to the FRONT of the bass instruction
    stream (see _hoist_input_dma): its ~1.1us descriptor generation and
    ~700ns ring launch run UNDER the bass preamble (register inits,
    const memsets, all-engine barrier) instead of after it. Measured
    -1.0us.

  * One single input DMA on the Scalar HWDGE queue. The two HWDGE
    queues (Sync/Scalar) contend for ONE shared descriptor-generation
    device with run-dependent grant order, so split variants (14/15,
    22/7, 24/5, SWDGE, both-on-one-engine) all measured equal or worse;
    the single DMA is also immune to the grant-order race. Scalar's
    compiler preamble also retires ~150ns before Sync's.

  * The store's desc-gen (on Sync, the other HWDGE queue) is gated on
    the INPUT DMA semaphore, not on the matmuls: DMA descriptors encode
    addresses only, and desc-gen (~700ns) + ring launch (~650ns) from
    that trigger put the first o_t read ~550ns after the DVE copy
    retires (input sem -> matmuls -> copy takes ~1170ns). This pulls
    the whole store launch latency off the tail. Partial-gating earlier
    is impossible: the input DMA posts its +16 increment as a single
    packet after its last descriptor.

Raw Bass (no TileContext): this walrus build rejects instructions that
carry more than one sync wait, and Tile's end-of-context drain
aggregates one wait per live semaphore. The manual chain below carries
at most one wait per instruction.

Measured: ~10.3us NEFF exec time (run-to-run noise ~±0.1us), vs 12.3us
baseline. Remaining time is ~6.0us compiler-emitted preamble (host
start event, engine config loads, barriers — outside kernel control),
~2.2us input launch+stream, ~2.1us store launch+tail.
"""

import numpy as np

N_STATE = 64
N_INPUT = 32
N_ALL = N_STATE + N_INPUT  # 96
T_FULL = 128
N_CORES = 8
TB = T_FULL // N_CORES  # 16 output blocks per core
K = N_ALL + 1 + TB  # 113 contraction rows
W_COLS = N_STATE + TB  # 80: [rhs | lhsT] packed along the free dim
RPP = 4  # K-rows packed per SBUF partition (640B bf16 elements)
KP = 29  # packed partitions: K-rows 4p..4p+3 share partition p

_PROGRAM_CACHE = {}


def _build_program():
    import concourse.bass as bass
    import concourse.mybir as mybir

    f32 = mybir.dt.float32
    bf16 = mybir.dt.bfloat16
    nc = bass.Bass("TRN2", debug=False)

    w_a = nc.dram_tensor("w_a", [KP, RPP * W_COLS], bf16, kind="ExternalInput")
    out_d = nc.dram_tensor("out", [TB, N_STATE], f32, kind="ExternalOutput")

    # Instructions are emitted straight into the main block (no nc.Block()):
    # the per-engine branch into a Block basic block costs ~400ns on the
    # critical path. Each engine executes only its own instructions, in
    # program order, so the semaphore chain below is unchanged.
    with (
        nc.sbuf_tensor([KP, RPP * W_COLS], bf16) as w_t,
        nc.psum_tensor([TB, N_STATE], f32) as acc,
        nc.sbuf_tensor([TB, N_STATE], f32) as o_t,
        nc.semaphore("dma_a") as dma_a,
        nc.semaphore("mm") as mm,
        nc.semaphore("dma_out") as dma_out,
    ):
        nc.scalar.dma_start(out=w_t[:], in_=w_a[:]).then_inc(dma_a, 16)

        nc.tensor.wait_ge(dma_a, 16)
        for i in range(RPP):
            inst = nc.tensor.matmul(
                acc[:],
                w_t[0:KP, W_COLS * i + N_STATE : W_COLS * (i + 1)],
                w_t[0:KP, W_COLS * i : W_COLS * i + N_STATE],
                start=(i == 0),
                stop=(i == RPP - 1),
            )
            if i == RPP - 1:
                inst.then_inc(mm, 1)
        nc.vector.wait_ge(mm, 1)
        nc.vector.tensor_copy(o_t[:], acc[:])
        # Store desc-gen gated on the input DMA, not on mm (see module
        # docstring). Nobody waits on dma_out; the runtime quiesces DMA
        # queues before output readback.
        nc.sync.wait_ge(dma_a, 16)
        nc.sync.dma_start(out=out_d[:], in_=o_t[:]).then_inc(dma_out, 16)

    _hoist_input_dma(nc)
    return nc


def _hoist_input_dma(nc):
    """Move the input InstDMACopy to the head of the instruction list.

    Bass's constructor pre-emits per-engine register inits, const-AP
    memsets, and a final all-engine barrier before the kernel body. The
    input load depends on none of that — its APs are static and its SBUF
    destination is untouched by the preamble — so hoisting it lets
    descriptor generation and the HWDGE ring launch run UNDER the
    preamble instead of after it, landing the input ~1.1us earlier. Each
    engine executes its own instructions in list order, so only the
    Scalar engine's relative order changes; every consumer of the data
    remains gated on the dma_a semaphore.
    """
    blk = nc.m.functions[0].blocks[0]
    insts = blk.instructions
    # the input load is the first DMA copy in program order (the store
    # is emitted last)
    dmas = [i for i in insts if type(i).__name__ == "InstDMACopy"][:1]
    assert len(dmas) == 1
    names = {i.name for i in dmas}
    rest = [i for i in insts if i.name not in names]
    blk.instructions = [rest[0], *dmas, *rest[1:]]
    got = [type(i).__name__ for i in blk.instructions[:3]]
    assert got[1] == "InstDMACopy", got


def _get_program():
    if "nc" not in _PROGRAM_CACHE:
        _PROGRAM_CACHE["nc"] = _build_program()
    return _PROGRAM_CACHE["nc"]


def _make_in_maps(A, B, x0, net_input):
    import ml_dtypes

    A = np.ascontiguousarray(A, dtype=np.float32)
    B = np.ascontiguousarray(B, dtype=np.float32)
    x0 = np.ascontiguousarray(x0, dtype=np.float32)
    V = np.ascontiguousarray(net_input, dtype=np.float32).reshape(T_FULL, N_ALL)

    ab_t = np.concatenate([A, B], axis=1).T  # (96, 64)

    in_maps = []
    for k in range(N_CORES):
        w = np.zeros((K, W_COLS), dtype=np.float32)
        rhs = w[:, :N_STATE]
        lhsT = w[:, N_STATE:]
        rhs[:N_ALL] = ab_t
        # rows 97..112: -I in lhsT, S rows in rhs
        lhsT[N_ALL + 1 :] = -np.eye(TB, dtype=np.float32)
        t0 = k * TB
        if k == 0:
            rhs[N_ALL] = V[0, :N_STATE]  # identity-block fixup
            lhsT[N_ALL, 0] = 1.0
            lhsT[:N_ALL, 1:] = V[0 : TB - 1].T
            rhs[N_ALL + 1] = x0
            rhs[N_ALL + 2 :] = V[1:TB, :N_STATE]
        else:
            lhsT[:N_ALL] = V[t0 - 1 : t0 + TB - 1].T
            rhs[N_ALL + 1 :] = V[t0 : t0 + TB, :N_STATE]
        # pack RPP K-rows per partition: partition p, col-group i <- row RPP*p+i
        w4 = np.zeros((KP * RPP, W_COLS), dtype=np.float32)
        w4[:K] = w
        w4 = w4.reshape(KP, RPP * W_COLS).astype(ml_dtypes.bfloat16)
        in_maps.append({"w_a": np.ascontiguousarray(w4)})
    return in_maps


def kernel(A, B, x0, net_input, T):
    assert int(T) == T_FULL, f"kernel hardcoded for T={T_FULL}, got {T}"
    from concourse.bass_utils import run_bass_kernel_spmd

    nc = _get_program()
    in_maps = _make_in_maps(A, B, x0, net_input)
    res = run_bass_kernel_spmd(nc, in_maps, core_ids=list(range(N_CORES)))
    out = np.concatenate([np.asarray(r["out"]).reshape(-1) for r in res.results])
    return out.astype(np.float32)
